# revision 1
# baseline (speedup 1.0000x reference)
"""CapsuleLayer (dynamic routing) Trainium2 Bass kernel.

Problem: x [64, 2048, 16], W [32, 2048, 32, 16] ->
  hat = einsum('bik,jidk->bijd', x, W); 3 routing iterations -> v [64, 32, 32].

Strategy (8 NeuronCores, In=2048 sharded 256/core; W never replicated):
  * hat is NEVER materialized. Three matmul families, all dense K=128:
      pass0:  s0 = (1/Nc) * sum_i hat  via big-K contraction over (i,k)
      (A):    agreement logits b += sum_d hat*v computed as
              G = (W . v) on PE (v folded into W), then DVE mult+reduce vs x
      (B):    s = sum_i c*hat computed as xc = c*x (DVE), DMA-transpose to
              (k,i)-partition layout, PE contraction vs W
  * s is AllReduced across cores ([128,8,64] fp32, 3x); squash computed
    redundantly on every core.
  * bf16 operands, fp32 PSUM accumulation / logits / s / v.

Layout conventions (per core, i_loc in [0,256)):
  j-map:   r = j%4, c2 = (j//4)%2, ga = j//8, jj = 4*(j//8)+j%4, slot = j//4
  ik-major ((A) path):      ik = i_loc*16 + k
  km-major ((B)/pass0):     km = k*256 + i_loc; K-tile t2 = km//128
  sT/vT canonical:          sT[32*(j%4)+d, j//4, b]
  logits:                   [(c2,b) partition, ga, r, i_loc]
"""
import sys

if "/opt/trn_rl_repo" not in sys.path:
    sys.path.insert(0, "/opt/trn_rl_repo")

from contextlib import ExitStack

import ml_dtypes
import numpy as np

import concourse.mybir as mybir
import concourse.tile as tile
from concourse import bacc
from concourse.bass_utils import run_bass_kernel_spmd

B, In, Din, Nc, Dc = 64, 2048, 16, 32, 32
NCORES = 8
IL = In // NCORES  # 256
EPS = 1e-7
FP32 = mybir.dt.float32
BF16 = mybir.dt.bfloat16

_KM_K = np.arange(4096) // 256   # km-major: k index
_KM_I = np.arange(4096) % 256    # km-major: i_loc index
_IK_I = np.arange(4096) // 16    # ik-major: i_loc index
_IK_K = np.arange(4096) % 16     # ik-major: k index


def _host_prep_core(x, W, core):
    """Build per-core input arrays. x, W are the full fp32 inputs."""
    i0 = core * IL
    Wc = np.ascontiguousarray(W[:, i0:i0 + IL])        # [Nc, IL, Dc, Din]
    xc = np.ascontiguousarray(x[:, i0:i0 + IL])        # [B, IL, Din]

    km = np.arange(128)[:, None] + 128 * np.arange(32)[None, :]   # [128, 32]
    ikm, kkm = _KM_I[km], _KM_K[km]

    # Wt [128, 32, 1024] bf16: Wt[p, t2, 32j+d] = W[j, i(km), d, k(km)]
    Wt = np.empty((128, 32, 1024), np.float32)
    for j in range(Nc):
        Wt[:, :, 32 * j:32 * j + 32] = Wc[j][ikm, :, kkm]
    # Wd [128, 8, 4096] bf16: Wd[32*(j%4)+d, j//4, ik] = W[j, i_ik, d, k_ik]
    Wd = np.empty((128, 8, 4096), np.float32)
    for j in range(Nc):
        Wd[32 * (j % 4):32 * (j % 4) + 32, j // 4, :] = Wc[j][_IK_I, :, _IK_K].T
    # xT [128, 32, 64] bf16 (pre-scaled 1/Nc): xT[p, t2, b] = x[b, i(km), k(km)]/Nc
    xT = (xc[:, ikm, kkm] / Nc).transpose(1, 2, 0)
    # x2a [(c2,b), ik] bf16 (same data both halves)
    xa = xc[:, _IK_I, _IK_K]                           # [B, 4096]
    x2a = np.concatenate([xa, xa], axis=0)             # [128, 4096]
    # xkT [km%128, t2, (c2,b)] bf16: host-transposed km-major x for (B)
    xk = xc[:, _KM_I, _KM_K]                           # [B, 4096]
    x2k = np.concatenate([xk, xk], axis=0)             # [128(c2,b), 4096]
    xkT = x2k.T.reshape(32, 128, 128).transpose(1, 0, 2)  # [128, 32, 128]

    bf = ml_dtypes.bfloat16
    return {
        "Wt": np.ascontiguousarray(Wt.astype(bf)),
        "Wd": np.ascontiguousarray(Wd.astype(bf)),
        "xT": np.ascontiguousarray(xT.astype(bf)),
        "x2a": np.ascontiguousarray(x2a.astype(bf)),
        "xkT": np.ascontiguousarray(xkT.astype(bf)),
        "consts": _host_consts(),
    }


def _host_consts():
    """[128, 292] fp32 const block:
      cols [0:128]   identity (PE transpose operand)
      cols [128:132] ones4 blockdiag: ones4[p, q] = (p//32 == q)
                     (partition-sum over d within a j-strip)
      cols [132:260] repM: repM[q, p] = (p//32 == q), used as [4, 128]
                     stationary to replicate a per-strip scalar over d
    """
    out = np.zeros((128, 292), np.float32)
    out[:, 0:128] = np.eye(128, dtype=np.float32)
    for q in range(4):
        out[32 * q:32 * q + 32, 128 + q] = 1.0
    for p in range(128):
        out[p // 32, 132 + p] = 1.0
    return np.ascontiguousarray(out)


def build_program(repeat=1):
    """Build the SPMD Bass/Tile program. repeat>1 duplicates the whole
    computation (for differential wall-clock timing)."""
    nc = bacc.Bacc("TRN2", target_bir_lowering=False, debug=False,
                   num_devices=NCORES)

    d_Wt = nc.dram_tensor("Wt", [128, 32, 1024], BF16, kind="ExternalInput").ap()
    d_Wd = nc.dram_tensor("Wd", [128, 8, 4096], BF16, kind="ExternalInput").ap()
    d_xT = nc.dram_tensor("xT", [128, 32, 64], BF16, kind="ExternalInput").ap()
    d_x2a = nc.dram_tensor("x2a", [128, 4096], BF16, kind="ExternalInput").ap()
    d_xkT = nc.dram_tensor("xkT", [128, 32, 128], BF16,
                           kind="ExternalInput").ap()
    d_cst = nc.dram_tensor("consts", [128, 292], FP32, kind="ExternalInput").ap()
    d_out = nc.dram_tensor("out", [128, 8, 64], FP32, kind="ExternalOutput").ap()

    cc_in = nc.dram_tensor("cc_in", [128, 8, 64], FP32).ap()
    cc_out = nc.dram_tensor("cc_out", [128, 8, 64], FP32, addr_space="Shared").ap()
    core_ids = list(range(NCORES))

    with tile.TileContext(nc) as tc, ExitStack() as ctx:
        ep = ctx.enter_context
        # ------------------------------------------------ pools
        p_const = ep(tc.tile_pool(name="const", bufs=1))
        p_wstream = ep(tc.tile_pool(name="wstream", bufs=3))
        p_wm = ep(tc.tile_pool(name="wm", bufs=2))
        p_small = ep(tc.tile_pool(name="small", bufs=1))
        p_gevac = ep(tc.tile_pool(name="gevac", bufs=3))
        p_prod = ep(tc.tile_pool(name="prod", bufs=2))
        p_red = ep(tc.tile_pool(name="red", bufs=3))
        p_eT = ep(tc.tile_pool(name="eT", bufs=4))
        p_xcT = ep(tc.tile_pool(name="xcT", bufs=4))
        # Single PSUM pool, one shared tag: slot = 4 banks, 2 slots = all 8.
        p_ps_g = ep(tc.tile_pool(name="ps_g", bufs=2, space="PSUM"))

        # ------------------------------------------------ resident tiles
        cst = p_const.tile([128, 292], FP32, tag="cst")
        nc.sync.dma_start(cst[:], d_cst)
        ident = cst[:, 0:128]
        ones4 = cst[:, 128:132]          # [128, 4]: blockdiag over d-strips
        repM = cst[0:4, 132:260]         # [4, 128]: scale replicate stationary

        xT = p_const.tile([128, 32, 64], BF16, tag="xT")
        x2a = p_const.tile([128, 4096], BF16, tag="x2a")
        xkT = p_const.tile([128, 32, 128], BF16, tag="xkT")
        nc.sync.dma_start(xT[:], d_xT)
        nc.sync.dma_start(x2a[:], d_x2a)
        nc.sync.dma_start(xkT[:], d_xkT)

        # bf16 logits: DVE reduce/add internal accum is fp32; one rounding per
        # pass. Keeps every (A) DVE op in the 2x perf mode and saves 8KB.
        logits = p_const.tile([128, 4, 4, 256], BF16, tag="logits")
        vT = p_const.tile([128, 8, 64], BF16, tag="vT")      # squash output
        sT_sb = p_const.tile([128, 8, 64], FP32, tag="sT_sb")
        e_t = p_const.tile([128, 16, 256], BF16, tag="e_t")  # exp(logits)
        zrow = p_const.tile([128, 256], FP32, tag="zrow")    # per-half sum

        for _rep in range(repeat):
            # ================================================ pass 0
            # s0[b, jd] accumulated over 32 km-tiles; moving = streamed Wt tile.
            ps_s0 = p_ps_g.tile([64, 1024], FP32, tag="ps")
            for t2a in range(16):
                wt_t = p_wstream.tile([128, 2, 1024], BF16, tag="wt_s")
                nc.sync.dma_start(wt_t[:], d_Wt[:, 2 * t2a:2 * t2a + 2, :])
                for tl in range(2):
                    t2 = 2 * t2a + tl
                    for half in range(2):
                        nc.tensor.matmul(
                            ps_s0[:, 512 * half:512 * half + 512],
                            xT[:, t2, :],                  # stationary [128, 64]
                            wt_t[:, tl, 512 * half:512 * half + 512],
                            start=(t2 == 0), stop=(t2 == 31),
                        )
            s0_sb = p_small.tile([64, 1024], FP32, tag="s0_sb")
            nc.vector.tensor_copy(s0_sb[:], ps_s0[:])
            # PE-transpose 8 blocks [64, 128] -> s0T psum [128, 8, 64]
            ps_s0T = p_ps_g.tile([128, 8, 64], FP32, tag="ps")
            for m in range(8):
                nc.tensor.transpose(ps_s0T[:, m, :], s0_sb[:, 128 * m:128 * m + 128],
                                    ident[0:64, 0:64])
            nc.vector.tensor_copy(sT_sb[:], ps_s0T[:])

            def allreduce_sT():
                nc.sync.dma_start(cc_in[:], sT_sb[:])
                nc.gpsimd.collective_compute(
                    "AllReduce", mybir.AluOpType.add,
                    replica_groups=[core_ids],
                    ins=[cc_in[:]], outs=[cc_out[:]],
                )
                nc.sync.dma_start(sT_sb[:], cc_out[:])

            def squash(out_bf16, out_fp32=None):
                """sT_sb [128,8,64] -> vT (bf16) and optionally fp32 copy.

                scale = s2/(1+s2) / sqrt(s2+eps); sqrt via ACT + one Newton step
                (ACT Sqrt table has a loose precision budget), divides via DVE
                bit-exact reciprocal.
                """
                sq = p_small.tile([128, 8, 64], FP32, tag="sq")
                nc.vector.tensor_tensor(sq[:], sT_sb[:], sT_sb[:],
                                        op=mybir.AluOpType.mult)
                ps_s2 = p_ps_g.tile([4, 8, 64], FP32, tag="ps")
                for slot in range(8):
                    nc.tensor.matmul(ps_s2[:, slot, :], ones4, sq[:, slot, :],
                                     start=True, stop=True)
                s2 = p_small.tile([4, 8, 64], FP32, tag="s2")
                nc.vector.tensor_copy(s2[:], ps_s2[:])
                t = p_small.tile([4, 8, 64], FP32, tag="t")
                nc.vector.tensor_scalar(t[:], s2[:], EPS, None,
                                        op0=mybir.AluOpType.add)
                y = p_small.tile([4, 8, 64], FP32, tag="y")
                nc.scalar.sqrt(y[:], t[:])
                # Newton for sqrt: y' = 0.5*(y + t/y)
                ry = p_small.tile([4, 8, 64], FP32, tag="ry")
                nc.vector.reciprocal(ry[:], y[:])
                nc.vector.tensor_tensor(ry[:], ry[:], t[:], op=mybir.AluOpType.mult)
                nc.vector.tensor_tensor(y[:], y[:], ry[:], op=mybir.AluOpType.add)
                nc.vector.tensor_scalar(y[:], y[:], 0.5, None,
                                        op0=mybir.AluOpType.mult)
                # den = (1+s2)*y ; scale = s2 * recip(den)
                den = p_small.tile([4, 8, 64], FP32, tag="den")
                nc.vector.tensor_scalar(den[:], s2[:], 1.0, None,
                                        op0=mybir.AluOpType.add)
                nc.vector.tensor_tensor(den[:], den[:], y[:], op=mybir.AluOpType.mult)
                nc.vector.reciprocal(den[:], den[:])
                scl = p_small.tile([4, 8, 64], FP32, tag="scl")
                nc.vector.tensor_tensor(scl[:], den[:], s2[:], op=mybir.AluOpType.mult)
                # replicate over d: ps_rep [128, 8, 64] = repM^T . scl
                ps_rep = p_ps_g.tile([128, 8, 64], FP32, tag="ps")
                for slot in range(8):
                    nc.tensor.matmul(ps_rep[:, slot, :], repM, scl[:, slot, :],
                                     start=True, stop=True)
                nc.vector.tensor_tensor(out_bf16[:], sT_sb[:], ps_rep[:],
                                        op=mybir.AluOpType.mult)
                if out_fp32 is not None:
                    nc.vector.tensor_tensor(out_fp32[:], sT_sb[:], ps_rep[:],
                                            op=mybir.AluOpType.mult)

            allreduce_sT()
            # fold the 1/Nc uniform-c scale: xT was pre-scaled on host.
            squash(vT)

            # ================================================ passes 1, 2
            for pas in range(2):
                # ---------------- (A): G = Wd . vT ; logits += sum_k x2a * G
                for ga in range(4):
                    for cha in range(4):
                        wd_t = p_wstream.tile([128, 2, 1024], BF16, tag="wd_s")
                        nc.sync.dma_start(wd_t[:],
                                          d_Wd[:, 2 * ga:2 * ga + 2,
                                               1024 * cha:1024 * cha + 1024])
                        for chl in range(2):
                            ch = 2 * cha + chl
                            ps_G = p_ps_g.tile([128, 4, 512], FP32, tag="ps")
                            for r in range(4):
                                for c2 in range(2):
                                    nc.tensor.matmul(
                                        ps_G[64 * c2:64 * c2 + 64, r, :],
                                        vT[32 * r:32 * r + 32, 2 * ga + c2, :],
                                        wd_t[32 * r:32 * r + 32, c2,
                                             512 * chl:512 * chl + 512],
                                        start=True, stop=True,
                                        tile_position=(32 * r, 64 * c2),
                                    )
                            # evac both chunk-halves into one double-width
                            # buffer; DVE then runs one wide unit per cha.
                            if chl == 0:
                                gev = p_gevac.tile([128, 4, 2, 512], BF16,
                                                   tag="gev")
                            nc.scalar.copy(gev[:, :, chl, :], ps_G[:])
                        prod = p_prod.tile([128, 4, 1024], BF16, tag="prod")
                        x2sl = x2a[:, 1024 * cha:1024 * cha + 1024]
                        nc.vector.tensor_tensor(
                            prod[:],
                            gev[:].rearrange("p r c f -> p r (c f)"),
                            x2sl.unsqueeze(1).broadcast_to((128, 4, 1024)),
                            op=mybir.AluOpType.mult)
                        # TensorReduce has no 2x uop (1x only): sum k=16 as
                        # a log-tree of in-place TT adds, all 2x-mode.
                        pv = prod[:].rearrange("p r (i k) -> p r i k", k=16)
                        for w in (8, 4, 2):
                            nc.vector.tensor_tensor(
                                pv[:, :, :, 0:w], pv[:, :, :, 0:w],
                                pv[:, :, :, w:2 * w],
                                op=mybir.AluOpType.add)
                        lsl = logits[:, ga, :, 64 * cha:64 * cha + 64]
                        # last tree level fused with the logits update
                        if pas == 0:
                            nc.vector.tensor_tensor(
                                lsl, pv[:, :, :, 0], pv[:, :, :, 1],
                                op=mybir.AluOpType.add)
                        else:
                            red = p_red.tile([128, 4, 64], BF16, tag="red")
                            nc.vector.tensor_tensor(
                                red[:], pv[:, :, :, 0], pv[:, :, :, 1],
                                op=mybir.AluOpType.add)
                            nc.vector.tensor_tensor(lsl, lsl, red[:],
                                                    op=mybir.AluOpType.add)
                # ---------------- softmax over j (split-j layout)
                nc.scalar.activation(e_t[:].rearrange("p a b -> p (a b)"),
                                     logits[:].rearrange("p g r i -> p (g r i)"),
                                     mybir.ActivationFunctionType.Exp)
                # Zh = sum over jj: tree of 2x-mode TT adds (reduce is 1x-only)
                esc = p_small.tile([128, 8, 256], BF16, tag="esc")
                nc.vector.tensor_tensor(esc[:], e_t[:, 0:8, :], e_t[:, 8:16, :],
                                        op=mybir.AluOpType.add)
                for w in (4, 2):
                    nc.vector.tensor_tensor(
                        esc[:, 0:w, :], esc[:, 0:w, :], esc[:, w:2 * w, :],
                        op=mybir.AluOpType.add)
                nc.vector.tensor_tensor(zrow[:], esc[:, 0, :], esc[:, 1, :],
                                        op=mybir.AluOpType.add)
                # cross-half add: copy upper half partitions down, add, recip,
                # then copy recip back up.
                ztmp = p_small.tile([64, 256], FP32, tag="ztmp")
                nc.sync.dma_start(ztmp[:], zrow[64:128, :])
                nc.vector.tensor_tensor(zrow[0:64, :], zrow[0:64, :], ztmp[:],
                                        op=mybir.AluOpType.add)
                rz = p_small.tile([128, 256], BF16, tag="rz")
                with nc.allow_low_precision("bf16 softmax 1/Z"):
                    nc.vector.reciprocal(rz[0:64, :], zrow[0:64, :])
                nc.sync.dma_start(rz[64:128, :], rz[0:64, :])
                # Build xc directly in the transposed (km-partition) layout:
                # only rz and e go through the DMA xbar (~1MB/pass, not the
                # 16.8MB xc itself). Block-transpose: out[p,ib,n]=in[n,128ib+p].
                rzT = p_small.tile([128, 2, 128], BF16, tag="rzT")
                nc.sync.dma_start(rzT[:], rz[:], transpose=True)
                # xrT[p, (k,ib), n] = xkT * rzT (rzT broadcast over k)
                xrT = p_small.tile([128, 32, 128], BF16, tag="xrT")
                nc.vector.tensor_tensor(
                    xrT[:].rearrange("p (k ib) n -> p k ib n", ib=2),
                    xkT[:].rearrange("p (k ib) n -> p k ib n", ib=2),
                    rzT[:].unsqueeze(1).broadcast_to((128, 16, 2, 128)),
                    op=mybir.AluOpType.mult)
                # ---------------- (B): xcT = xrT * e_jj^T -> PE contraction
                last = (pas == 1)
                ps_sT = p_ps_g.tile([128, 8, 64], FP32, tag="ps")
                for m in range(4):
                    xcT_bufs = []
                    for jq in range(4):
                        jj = 4 * m + jq
                        eT = p_eT.tile([128, 2, 128], BF16, tag="eT")
                        nc.sync.dma_start(eT[:], e_t[:, jj, :], transpose=True)
                        xcT = p_xcT.tile([128, 32, 128], BF16, tag="xcT")
                        nc.vector.tensor_tensor(
                            xcT[:].rearrange("p (k ib) n -> p k ib n", ib=2),
                            xrT[:].rearrange("p (k ib) n -> p k ib n", ib=2),
                            eT[:].unsqueeze(1).broadcast_to((128, 16, 2, 128)),
                            op=mybir.AluOpType.mult)
                        xcT_bufs.append(xcT)
                    # stationary slab for all 8 j's of this m, all t2
                    wtm = p_wm.tile([128, 32, 256], BF16, tag="wtm")
                    nc.sync.dma_start(wtm[:], d_Wt[:, :, 256 * m:256 * m + 256])
                    # t2 INNERMOST: each accumulation group completes before the
                    # next starts (start=True clears has_written bank-wide).
                    for gq in (2 * m, 2 * m + 1):
                        c2 = gq % 2
                        for q in range(4):
                            j = 4 * gq + q           # j%4 == q, jj = 4*m + q
                            jl = j - 8 * m
                            for t2 in range(32):
                                nc.tensor.matmul(
                                    ps_sT[32 * q:32 * q + 32, gq, :],
                                    wtm[:, t2, 32 * jl:32 * jl + 32],
                                    xcT_bufs[q][:, t2, 64 * c2:64 * c2 + 64],
                                    start=(t2 == 0), stop=(t2 == 31),
                                    tile_position=(0, 32 * q),
                                    skip_group_check=True,
                                )
                nc.vector.tensor_copy(sT_sb[:], ps_sT[:])
                allreduce_sT()
                if not last:
                    squash(vT)
                else:
                    vfin = p_small.tile([128, 8, 64], FP32, tag="vfin")
                    squash(vT, out_fp32=vfin)
                    nc.sync.dma_start(d_out, vfin[:])

    nc.compile()
    return nc


def kernel(x, W):
    x = np.asarray(x, dtype=np.float32)
    W = np.asarray(W, dtype=np.float32)
    in_maps = [_host_prep_core(x, W, c) for c in range(NCORES)]

    nc = build_program()
    res = run_bass_kernel_spmd(nc, in_maps, list(range(NCORES)))
    vT = res.results[0]["out"]  # [128, 8, 64]

    v = np.empty((B, Nc, Dc), np.float32)
    for j in range(Nc):
        v[:, j, :] = vT[32 * (j % 4):32 * (j % 4) + 32, j // 4, :].T
    return v


if __name__ == "__main__":
    rng = np.random.default_rng(0)
    x = rng.standard_normal((B, In, Din), dtype=np.float32)
    W = (rng.standard_normal((Nc, In, Dc, Din), dtype=np.float32) * 0.05)
    out = kernel(x, W)
    print("kernel ran; out shape", out.shape, "mean", float(np.abs(out).mean()))



# revision 16
# speedup vs baseline: 1.4926x; 1.4926x over previous
"""CapsuleLayer (dynamic routing) Trainium2 Bass kernel.

Problem: x [64, 2048, 16], W [32, 2048, 32, 16] ->
  hat = einsum('bik,jidk->bijd', x, W); 3 routing iterations -> v [64, 32, 32].

Strategy (8 NeuronCores, In=2048 sharded 256/core; W never replicated):
  * hat is NEVER materialized. Three matmul families, all dense K=128:
      pass0:  s0 = (1/Nc) * sum_i hat  via big-K contraction over (i,k)
      (A):    agreement logits b += sum_d hat*v computed as
              G = (W . v) on PE (v folded into W), then DVE mult+reduce vs x
      (B):    s = sum_i c*hat computed as xc = c*x (DVE), DMA-transpose to
              (k,i)-partition layout, PE contraction vs resident Wt
  * Wt ([128,32,1024] bf16, 8.4MB) is streamed from HBM once during pass0
    (alternating both HWDGE queues) and stays RESIDENT for both (B) passes
    (saves 16.8MB of HBM re-streaming). Wd streams per (A) pass with a
    4-deep prefetch pool so transfers ride the idle AllReduce windows.
  * s is AllReduced across cores in bf16 ([128,8,64], 3x, ~30us fixed cost
    each on this runtime — remote_dma is unsupported here and chunking
    loses: the cost is size-insensitive). squash computed redundantly on
    every core.
  * 3 of 16 (A) iterations per pass (mult + k-reduce tree + logit update)
    run entirely on the otherwise-idle Pool engine; DVE keeps the rest.
  * bf16 operands, fp32 PSUM accumulation; s/logits bf16, final v fp32.

Layout conventions (per core, i_loc in [0,256)):
  j-map:   r = j%4, c2 = (j//4)%2, ga = j//8, jj = 4*(j//8)+j%4, slot = j//4
  ik-major ((A) path):      ik = i_loc*16 + k
  km-major ((B)/pass0):     km = k*256 + i_loc; K-tile t2 = km//128
  sT/vT canonical:          sT[32*(j%4)+d, j//4, b]
  logits:                   [(c2,b) partition, ga, r, i_loc]
"""
import sys

if "/opt/trn_rl_repo" not in sys.path:
    sys.path.insert(0, "/opt/trn_rl_repo")

from contextlib import ExitStack

import ml_dtypes
import numpy as np

import concourse.mybir as mybir
import concourse.tile as tile
from concourse import bacc
from concourse.bass_utils import run_bass_kernel_spmd

B, In, Din, Nc, Dc = 64, 2048, 16, 32, 32
NCORES = 8
IL = In // NCORES  # 256
EPS = 1e-7
FP32 = mybir.dt.float32
BF16 = mybir.dt.bfloat16

_KM_K = np.arange(4096) // 256   # km-major: k index
_KM_I = np.arange(4096) % 256    # km-major: i_loc index
_IK_I = np.arange(4096) // 16    # ik-major: i_loc index
_IK_K = np.arange(4096) % 16     # ik-major: k index


def _host_prep_core(x, W, core):
    """Build per-core input arrays. x, W are the full fp32 inputs."""
    i0 = core * IL
    Wc = np.ascontiguousarray(W[:, i0:i0 + IL])        # [Nc, IL, Dc, Din]
    xc = np.ascontiguousarray(x[:, i0:i0 + IL])        # [B, IL, Din]

    km = np.arange(128)[:, None] + 128 * np.arange(32)[None, :]   # [128, 32]
    ikm, kkm = _KM_I[km], _KM_K[km]

    # Wt [128, 32, 1024] bf16: Wt[p, t2, 32j+d] = W[j, i(km), d, k(km)]
    Wt = np.empty((128, 32, 1024), np.float32)
    for j in range(Nc):
        Wt[:, :, 32 * j:32 * j + 32] = Wc[j][ikm, :, kkm]
    # Wd [128, 8, 4096] bf16: Wd[32*(j%4)+d, j//4, ik] = W[j, i_ik, d, k_ik]
    Wd = np.empty((128, 8, 4096), np.float32)
    for j in range(Nc):
        Wd[32 * (j % 4):32 * (j % 4) + 32, j // 4, :] = Wc[j][_IK_I, :, _IK_K].T
    # xT [128, 32, 64] bf16 (pre-scaled 1/Nc): xT[p, t2, b] = x[b, i(km), k(km)]/Nc
    xT = (xc[:, ikm, kkm] / Nc).transpose(1, 2, 0)
    # x2a [(c2,b), ik] bf16 (same data both halves)
    xa = xc[:, _IK_I, _IK_K]                           # [B, 4096]
    x2a = np.concatenate([xa, xa], axis=0)             # [128, 4096]
    # xkT [km%128, t2, (c2,b)] bf16: host-transposed km-major x for (B)
    xk = xc[:, _KM_I, _KM_K]                           # [B, 4096]
    x2k = np.concatenate([xk, xk], axis=0)             # [128(c2,b), 4096]
    xkT = x2k.T.reshape(32, 128, 128).transpose(1, 0, 2)  # [128, 32, 128]

    bf = ml_dtypes.bfloat16
    return {
        "Wt": np.ascontiguousarray(Wt.astype(bf)),
        "Wd": np.ascontiguousarray(Wd.astype(bf)),
        "xT": np.ascontiguousarray(xT.astype(bf)),
        "x2a": np.ascontiguousarray(x2a.astype(bf)),
        "xkT": np.ascontiguousarray(xkT.astype(bf)),
        "consts": _host_consts(),
    }


def _host_consts():
    """[128, 292] fp32 const block:
      cols [0:128]   identity (PE transpose operand)
      cols [128:132] ones4 blockdiag: ones4[p, q] = (p//32 == q)
                     (partition-sum over d within a j-strip)
      cols [132:260] repM: repM[q, p] = (p//32 == q), used as [4, 128]
                     stationary to replicate a per-strip scalar over d
    """
    out = np.zeros((128, 292), np.float32)
    out[:, 0:128] = np.eye(128, dtype=np.float32)
    for q in range(4):
        out[32 * q:32 * q + 32, 128 + q] = 1.0
    for p in range(128):
        out[p // 32, 132 + p] = 1.0
    return np.ascontiguousarray(out)


def build_program(repeat=1):
    """Build the SPMD Bass/Tile program. repeat>1 duplicates the whole
    computation (for differential wall-clock timing)."""
    nc = bacc.Bacc("TRN2", target_bir_lowering=False, debug=False,
                   num_devices=NCORES)

    d_Wt = nc.dram_tensor("Wt", [128, 32, 1024], BF16, kind="ExternalInput").ap()
    d_Wd = nc.dram_tensor("Wd", [128, 8, 4096], BF16, kind="ExternalInput").ap()
    d_xT = nc.dram_tensor("xT", [128, 32, 64], BF16, kind="ExternalInput").ap()
    d_x2a = nc.dram_tensor("x2a", [128, 4096], BF16, kind="ExternalInput").ap()
    d_xkT = nc.dram_tensor("xkT", [128, 32, 128], BF16,
                           kind="ExternalInput").ap()
    d_cst = nc.dram_tensor("consts", [128, 292], FP32, kind="ExternalInput").ap()
    d_out = nc.dram_tensor("out", [128, 8, 64], FP32, kind="ExternalOutput").ap()

    cc_in = nc.dram_tensor("cc_in", [128, 8, 64], BF16).ap()
    cc_out = nc.dram_tensor("cc_out", [128, 8, 64], BF16, addr_space="Shared").ap()
    core_ids = list(range(NCORES))

    with tile.TileContext(nc) as tc, ExitStack() as ctx:
        ep = ctx.enter_context
        # ------------------------------------------------ pools
        p_const = ep(tc.tile_pool(name="const", bufs=1))
        p_wstream = ep(tc.tile_pool(name="wstream", bufs=4))
        p_small = ep(tc.tile_pool(name="small", bufs=1))
        p_gevac = ep(tc.tile_pool(name="gevac", bufs=2))
        p_prod = ep(tc.tile_pool(name="prod", bufs=2))
        p_red = ep(tc.tile_pool(name="red", bufs=2))
        p_eT = ep(tc.tile_pool(name="eT", bufs=2))
        p_xcT = ep(tc.tile_pool(name="xcT", bufs=2))
        # Single PSUM pool, one shared tag: slot = 4 banks, 2 slots = all 8.
        p_ps_g = ep(tc.tile_pool(name="ps_g", bufs=2, space="PSUM"))

        # ------------------------------------------------ resident tiles
        cst = p_const.tile([128, 292], FP32, tag="cst")
        nc.scalar.dma_start(cst[:], d_cst)
        ident = cst[:, 0:128]
        ones4 = cst[:, 128:132]          # [128, 4]: blockdiag over d-strips
        repM = cst[0:4, 132:260]         # [4, 128]: scale replicate stationary

        xT = p_const.tile([128, 32, 64], BF16, tag="xT")
        x2a = p_const.tile([128, 4096], BF16, tag="x2a")
        xkT = p_const.tile([128, 32, 128], BF16, tag="xkT")
        nc.sync.dma_start(xT[:], d_xT)
        nc.scalar.dma_start(x2a[:], d_x2a)
        nc.scalar.dma_start(xkT[:], d_xkT)

        # Wt resident: streamed in during pass0, reused by (B) in both passes
        # (saves 2x 8.4MB of HBM re-streaming).
        Wt = p_const.tile([128, 32, 1024], BF16, tag="Wt_res")

        # bf16 logits: DVE reduce/add internal accum is fp32; one rounding per
        # pass. Keeps every (A) DVE op in the 2x perf mode and saves 8KB.
        logits = p_const.tile([128, 4, 4, 256], BF16, tag="logits")
        vT = p_const.tile([128, 8, 64], BF16, tag="vT")      # squash output
        sTh = p_const.tile([128, 8, 64], BF16, tag="sTh")    # s (bf16, AR'd)
        e_t = p_const.tile([128, 16, 256], BF16, tag="e_t")  # exp(logits)
        zrow = p_const.tile([128, 256], FP32, tag="zrow")    # per-half sum

        for _rep in range(repeat):
            # ================================================ pass 0
            # s0[b, jd] accumulated over 32 km-tiles; moving = streamed Wt tile.
            ps_s0 = p_ps_g.tile([64, 1024], FP32, tag="ps")
            for t2a in range(16):
                # alternate the two HWDGE queues (SP / ACT) for 2x stream bw
                qeng = nc.sync if t2a % 2 == 0 else nc.scalar
                qeng.dma_start(Wt[:, 2 * t2a:2 * t2a + 2, :],
                               d_Wt[:, 2 * t2a:2 * t2a + 2, :])
                for tl in range(2):
                    t2 = 2 * t2a + tl
                    for half in range(2):
                        nc.tensor.matmul(
                            ps_s0[:, 512 * half:512 * half + 512],
                            xT[:, t2, :],                  # stationary [128, 64]
                            Wt[:, t2, 512 * half:512 * half + 512],
                            start=(t2 == 0), stop=(t2 == 31),
                        )
            s0_sb = p_small.tile([64, 1024], FP32, tag="s0_sb")
            nc.vector.tensor_copy(s0_sb[:], ps_s0[:])
            # PE-transpose 8 blocks [64, 128] -> s0T psum [128, 8, 64]
            ps_s0T = p_ps_g.tile([128, 8, 64], FP32, tag="ps")
            for m in range(8):
                nc.tensor.transpose(ps_s0T[:, m, :], s0_sb[:, 128 * m:128 * m + 128],
                                    ident[0:64, 0:64])
            nc.vector.tensor_copy(sTh[:], ps_s0T[:])

            def allreduce_sT():
                nc.sync.dma_start(cc_in[:], sTh[:])
                nc.gpsimd.collective_compute(
                    "AllReduce", mybir.AluOpType.add,
                    replica_groups=[core_ids],
                    ins=[cc_in[:]], outs=[cc_out[:]],
                )
                nc.sync.dma_start(sTh[:], cc_out[:])

            def squash(out_bf16, out_fp32=None):
                """sTh [128,8,64] bf16 -> vT (bf16) and optionally fp32 copy.

                scale = s2/(1+s2) / sqrt(s2+eps); sqrt via ACT + one Newton step
                (ACT Sqrt table has a loose precision budget), divides via DVE
                bit-exact reciprocal.
                """
                sq = p_small.tile([128, 8, 64], FP32, tag="sq")
                nc.vector.tensor_tensor(sq[:], sTh[:], sTh[:],
                                        op=mybir.AluOpType.mult)
                ps_s2 = p_ps_g.tile([4, 8, 64], FP32, tag="ps")
                for slot in range(8):
                    nc.tensor.matmul(ps_s2[:, slot, :], ones4, sq[:, slot, :],
                                     start=True, stop=True)
                s2 = p_small.tile([4, 8, 64], FP32, tag="s2")
                nc.vector.tensor_copy(s2[:], ps_s2[:])
                t = p_small.tile([4, 8, 64], FP32, tag="t")
                nc.vector.tensor_scalar(t[:], s2[:], EPS, None,
                                        op0=mybir.AluOpType.add)
                y = p_small.tile([4, 8, 64], FP32, tag="y")
                nc.scalar.sqrt(y[:], t[:])
                # Newton for sqrt: y' = 0.5*(y + t/y)
                ry = p_small.tile([4, 8, 64], FP32, tag="ry")
                nc.vector.reciprocal(ry[:], y[:])
                nc.vector.tensor_tensor(ry[:], ry[:], t[:], op=mybir.AluOpType.mult)
                nc.vector.tensor_tensor(y[:], y[:], ry[:], op=mybir.AluOpType.add)
                nc.vector.tensor_scalar(y[:], y[:], 0.5, None,
                                        op0=mybir.AluOpType.mult)
                # den = (1+s2)*y ; scale = s2 * recip(den)
                den = p_small.tile([4, 8, 64], FP32, tag="den")
                nc.vector.tensor_scalar(den[:], s2[:], 1.0, None,
                                        op0=mybir.AluOpType.add)
                nc.vector.tensor_tensor(den[:], den[:], y[:], op=mybir.AluOpType.mult)
                nc.vector.reciprocal(den[:], den[:])
                scl = p_small.tile([4, 8, 64], FP32, tag="scl")
                nc.vector.tensor_tensor(scl[:], den[:], s2[:], op=mybir.AluOpType.mult)
                # replicate over d: ps_rep [128, 8, 64] = repM^T . scl
                ps_rep = p_ps_g.tile([128, 8, 64], FP32, tag="ps")
                for slot in range(8):
                    nc.tensor.matmul(ps_rep[:, slot, :], repM, scl[:, slot, :],
                                     start=True, stop=True)
                nc.vector.tensor_tensor(out_bf16[:], sTh[:], ps_rep[:],
                                        op=mybir.AluOpType.mult)
                if out_fp32 is not None:
                    nc.vector.tensor_tensor(out_fp32[:], sTh[:], ps_rep[:],
                                            op=mybir.AluOpType.mult)

            allreduce_sT()
            # fold the 1/Nc uniform-c scale: xT was pre-scaled on host.
            squash(vT)

            # ================================================ passes 1, 2
            for pas in range(2):
                # ---------------- (A): G = Wd . vT ; logits += sum_k x2a * G
                for ga in range(4):
                    for cha in range(4):
                        wd_t = p_wstream.tile([128, 2, 1024], BF16, tag="wd_s")
                        nc.sync.dma_start(wd_t[:],
                                          d_Wd[:, 2 * ga:2 * ga + 2,
                                               1024 * cha:1024 * cha + 1024])
                        for chl in range(2):
                            ch = 2 * cha + chl
                            ps_G = p_ps_g.tile([128, 4, 512], FP32, tag="ps")
                            for r in range(4):
                                for c2 in range(2):
                                    nc.tensor.matmul(
                                        ps_G[64 * c2:64 * c2 + 64, r, :],
                                        vT[32 * r:32 * r + 32, 2 * ga + c2, :],
                                        wd_t[32 * r:32 * r + 32, c2,
                                             512 * chl:512 * chl + 512],
                                        start=True, stop=True,
                                        tile_position=(32 * r, 64 * c2),
                                    )
                            # evac both chunk-halves into one double-width
                            # buffer; DVE then runs one wide unit per cha.
                            if chl == 0:
                                gev = p_gevac.tile([128, 4, 2, 512], BF16,
                                                   tag="gev")
                            nc.scalar.copy(gev[:, :, chl, :], ps_G[:])
                        prod = p_prod.tile([128, 4, 1024], BF16, tag="prod")
                        x2sl = x2a[:, 1024 * cha:1024 * cha + 1024]
                        # Offload 3 of 16 iterations entirely to the (idle)
                        # Pool engine; DVE stays the pole but ~20% shorter.
                        ve = nc.gpsimd if (cha == 3 and ga <= 2) else nc.vector
                        ve.tensor_tensor(
                            prod[:],
                            gev[:].rearrange("p r c f -> p r (c f)"),
                            x2sl.unsqueeze(1).broadcast_to((128, 4, 1024)),
                            op=mybir.AluOpType.mult)
                        # TensorReduce has no 2x uop (1x only): sum k=16 as
                        # a log-tree of in-place TT adds, all 2x-mode.
                        pv = prod[:].rearrange("p r (i k) -> p r i k", k=16)
                        for w in (8, 4, 2):
                            ve.tensor_tensor(
                                pv[:, :, :, 0:w], pv[:, :, :, 0:w],
                                pv[:, :, :, w:2 * w],
                                op=mybir.AluOpType.add)
                        lsl = logits[:, ga, :, 64 * cha:64 * cha + 64]
                        # last tree level fused with the logits update
                        if pas == 0:
                            ve.tensor_tensor(
                                lsl, pv[:, :, :, 0], pv[:, :, :, 1],
                                op=mybir.AluOpType.add)
                        else:
                            red = p_red.tile([128, 4, 64], BF16, tag="red")
                            ve.tensor_tensor(
                                red[:], pv[:, :, :, 0], pv[:, :, :, 1],
                                op=mybir.AluOpType.add)
                            ve.tensor_tensor(lsl, lsl, red[:],
                                             op=mybir.AluOpType.add)
                # ---------------- softmax over j (split-j layout)
                nc.scalar.activation(e_t[:].rearrange("p a b -> p (a b)"),
                                     logits[:].rearrange("p g r i -> p (g r i)"),
                                     mybir.ActivationFunctionType.Exp)
                # Zh = sum over jj: tree of 2x-mode TT adds (reduce is 1x-only)
                esc = p_small.tile([128, 8, 256], BF16, tag="esc")
                nc.vector.tensor_tensor(esc[:], e_t[:, 0:8, :], e_t[:, 8:16, :],
                                        op=mybir.AluOpType.add)
                for w in (4, 2):
                    nc.vector.tensor_tensor(
                        esc[:, 0:w, :], esc[:, 0:w, :], esc[:, w:2 * w, :],
                        op=mybir.AluOpType.add)
                nc.vector.tensor_tensor(zrow[:], esc[:, 0, :], esc[:, 1, :],
                                        op=mybir.AluOpType.add)
                # cross-half add: copy upper half partitions down, add, recip,
                # then copy recip back up.
                ztmp = p_small.tile([64, 256], FP32, tag="ztmp")
                nc.sync.dma_start(ztmp[:], zrow[64:128, :])
                nc.vector.tensor_tensor(zrow[0:64, :], zrow[0:64, :], ztmp[:],
                                        op=mybir.AluOpType.add)
                rz = p_small.tile([128, 256], BF16, tag="rz")
                with nc.allow_low_precision("bf16 softmax 1/Z"):
                    nc.vector.reciprocal(rz[0:64, :], zrow[0:64, :])
                nc.sync.dma_start(rz[64:128, :], rz[0:64, :])
                # Build xc directly in the transposed (km-partition) layout:
                # only rz and e go through the DMA xbar (~1MB/pass, not the
                # 16.8MB xc itself). Block-transpose: out[p,ib,n]=in[n,128ib+p].
                rzT = p_small.tile([128, 2, 128], BF16, tag="rzT")
                nc.sync.dma_start(rzT[:], rz[:], transpose=True)
                # xrT[p, (k,ib), n] = xkT * rzT (rzT broadcast over k)
                xrT = p_small.tile([128, 32, 128], BF16, tag="xrT")
                nc.vector.tensor_tensor(
                    xrT[:].rearrange("p (k ib) n -> p k ib n", ib=2),
                    xkT[:].rearrange("p (k ib) n -> p k ib n", ib=2),
                    rzT[:].unsqueeze(1).broadcast_to((128, 16, 2, 128)),
                    op=mybir.AluOpType.mult)
                # ---------------- (B): xcT = xrT * e_jj^T -> PE contraction
                last = (pas == 1)
                ps_sT = p_ps_g.tile([128, 8, 64], FP32, tag="ps")
                for m in range(4):
                    for jq in range(4):
                        jj = 4 * m + jq
                        eT = p_eT.tile([128, 2, 128], BF16, tag="eT")
                        nc.sync.dma_start(eT[:], e_t[:, jj, :], transpose=True)
                        xcT = p_xcT.tile([128, 32, 128], BF16, tag="xcT")
                        nc.vector.tensor_tensor(
                            xcT[:].rearrange("p (k ib) n -> p k ib n", ib=2),
                            xrT[:].rearrange("p (k ib) n -> p k ib n", ib=2),
                            eT[:].unsqueeze(1).broadcast_to((128, 16, 2, 128)),
                            op=mybir.AluOpType.mult)
                        # t2 INNERMOST: each accumulation group completes
                        # before the next starts (start=True clears
                        # has_written bank-wide).
                        for c2 in range(2):
                            gq = 2 * m + c2
                            j = 4 * gq + jq          # j%4 == jq, jj = 4*m + jq
                            for t2 in range(32):
                                nc.tensor.matmul(
                                    ps_sT[32 * jq:32 * jq + 32, gq, :],
                                    Wt[:, t2, 32 * j:32 * j + 32],
                                    xcT[:, t2, 64 * c2:64 * c2 + 64],
                                    start=(t2 == 0), stop=(t2 == 31),
                                    tile_position=(0, 32 * jq),
                                    skip_group_check=True,
                                )
                nc.vector.tensor_copy(sTh[:], ps_sT[:])
                allreduce_sT()
                if not last:
                    squash(vT)
                else:
                    vfin = p_small.tile([128, 8, 64], FP32, tag="vfin")
                    squash(vT, out_fp32=vfin)
                    nc.sync.dma_start(d_out, vfin[:])

    nc.compile()
    return nc


def kernel(x, W):
    x = np.asarray(x, dtype=np.float32)
    W = np.asarray(W, dtype=np.float32)
    in_maps = [_host_prep_core(x, W, c) for c in range(NCORES)]

    nc = build_program()
    res = run_bass_kernel_spmd(nc, in_maps, list(range(NCORES)))
    vT = res.results[0]["out"]  # [128, 8, 64]

    v = np.empty((B, Nc, Dc), np.float32)
    for j in range(Nc):
        v[:, j, :] = vT[32 * (j % 4):32 * (j % 4) + 32, j // 4, :].T
    return v


if __name__ == "__main__":
    rng = np.random.default_rng(0)
    x = rng.standard_normal((B, In, Din), dtype=np.float32)
    W = (rng.standard_normal((Nc, In, Dc, Din), dtype=np.float32) * 0.05)
    out = kernel(x, W)
    print("kernel ran; out shape", out.shape, "mean", float(np.abs(out).mean()))

